# revision 23
# baseline (speedup 1.0000x reference)
"""Trainium2 kernel for nn_CustomModel_71227737637112 (Hungarian-matching loss).

reference semantics:
    dist[b,i,j] = || y_true[b,i,:] - y_pred[b,j,:] ||_2          [B=64, N=128, N]
    col = linear_sum_assignment(dist[b])  (host, per batch)
    loss = mean_b sum_i dist[b, i, col[b,i]]

Device part (8 NeuronCores, batch-sharded data parallel, 8 batches/core):
    dist^2 = |yt_i|^2 + |yp_j|^2 - 2 yt_i.yp_j as ONE K=68 bf16 matmul per
    batch.  The host pre-packs the (transposed, d-major) operand image
        lhsT[k,b,i] = [bf16(yt) | 1 | 1 | nt_hi | nt_lo]
        rhs [k,b,j] = [bf16(-2 yp) | np_hi | np_lo | 1 | 1]
    with both norms carried as bf16 value+residual row pairs (~f32
    accuracy).  The device then only does: one contiguous DMA in, 8
    matmuls (PE), 8 PSUM->SBUF copies (DVE/ACT alternating), chunked DMA
    out of dist^2.  Host: sqrt, LSAP (the reference also runs LSAP on host
    via pure_callback), gather + mean.
"""

import numpy as np

import concourse.bacc as bacc
import concourse.bass as bass
import concourse.mybir as mybir
from concourse.bass_utils import run_bass_kernel_spmd
from concourse.tile import TileContext

N_CORES = 8
B, N, D = 64, 128, 64
BPC = B // N_CORES  # batches per core
K = D + 4  # contraction: 64 data rows + np_hi/np_lo + nt_hi/nt_lo vs ones

FP32 = mybir.dt.float32
FP16 = mybir.dt.float16


def _build_nc() -> bass.Bass:
    nc = bacc.Bacc("TRN2", target_bir_lowering=False)
    # Pre-packed transposed operands, pair-major for chunked contiguous DMA:
    # [pair, k, side (0=lhsT yt, 1=rhs yp), b within pair, point]
    tta = nc.dram_tensor("tta", [BPC // 2, K, 2, 2, N], FP16, kind="ExternalInput")
    dist2 = nc.dram_tensor("dist2", [BPC, N, N], FP16, kind="ExternalOutput")

    with TileContext(nc) as tc:
        with (
            tc.tile_pool(name="sbuf", bufs=1) as sbuf,
            tc.tile_pool(name="psum_mm", bufs=4, space="PSUM") as psum_mm,
        ):
            tt = sbuf.tile([K, 2, BPC, N], FP16)
            d2 = sbuf.tile([N, BPC, N], FP16)

            # chunked input: matmuls on pair p start as soon as chunk p lands
            tt_pair = tt[:, :, :, :].rearrange("k s (p c) n -> p k s c n", c=2)
            for p in range(BPC // 2):
                nc.sync.dma_start(out=tt_pair[p], in_=tta[p, :, :, :, :])

            for b in range(BPC):
                mm = psum_mm.tile([N, N], FP32, tag="mm")
                nc.tensor.matmul(
                    mm[:, :], tt[:, 0, b, :], tt[:, 1, b, :], start=True, stop=True
                )
                # PSUM -> SBUF f16 copy, alternating engines to keep PE pace
                if b % 2 == 0:
                    nc.vector.tensor_copy(d2[:, b, :], mm[:, :])
                else:
                    nc.scalar.copy(d2[:, b, :], mm[:, :])
                nc.sync.dma_start(
                    out=dist2[b, :, :], in_=d2[:, b, :]
                )

    nc.finalize()
    return nc


_NC_CACHE = None


def _get_nc():
    global _NC_CACHE
    if _NC_CACHE is None:
        _NC_CACHE = _build_nc()
    return _NC_CACHE


# ---------------------------------------------------------------------------
# Host side: batched linear sum assignment (Hungarian).  The reference runs
# this on host through jax.pure_callback; we do the same.  scipy if present,
# else a vectorized Jonker-Volgenant implementation identical to the
# reference algorithm.
# ---------------------------------------------------------------------------


def _lsap_np(cost):
    cost = np.asarray(cost, dtype=np.float64)
    n = cost.shape[0]
    u = np.zeros(n + 1)
    v = np.zeros(n + 1)
    p = np.zeros(n + 1, dtype=np.int64)
    way = np.zeros(n + 1, dtype=np.int64)
    for i in range(1, n + 1):
        p[0] = i
        j0 = 0
        minv = np.full(n + 1, np.inf)
        used = np.zeros(n + 1, dtype=bool)
        while True:
            used[j0] = True
            i0 = p[j0]
            js = np.nonzero(~used[1:])[0] + 1
            cur = cost[i0 - 1, js - 1] - u[i0] - v[js]
            better = cur < minv[js]
            minv[js] = np.where(better, cur, minv[js])
            way[js] = np.where(better, j0, way[js])
            j1 = js[np.argmin(minv[js])]
            delta = minv[j1]
            u[p[used]] += delta
            v[used] -= delta
            minv[~used] -= delta
            j0 = j1
            if p[j0] == 0:
                break
        while j0 != 0:
            j1 = way[j0]
            p[j0] = p[j1]
            j0 = j1
    col_of_row = np.zeros(n, dtype=np.int32)
    for j in range(1, n + 1):
        if p[j] > 0:
            col_of_row[p[j] - 1] = j - 1
    return col_of_row


def _batched_lsap(dists):
    try:
        from scipy.optimize import linear_sum_assignment

        cols = np.empty((dists.shape[0], dists.shape[1]), dtype=np.int32)
        for b in range(dists.shape[0]):
            _, c = linear_sum_assignment(dists[b].astype(np.float64))
            cols[b] = c.astype(np.int32)
        return cols
    except Exception:
        return np.stack([_lsap_np(d) for d in dists]).astype(np.int32)


def _in_maps(y_true, y_pred):
    f16 = np.float16
    maps = []
    for c in range(N_CORES):
        yt = y_true[c * BPC : (c + 1) * BPC]  # [8, 128, 64] f32
        yp = y_pred[c * BPC : (c + 1) * BPC]
        nt = (yt.astype(np.float64) ** 2).sum(-1).astype(np.float32)  # [8, 128]
        npv = (yp.astype(np.float64) ** 2).sum(-1).astype(np.float32)
        nt_hi = nt.astype(f16)
        nt_lo = (nt - nt_hi.astype(np.float32)).astype(f16)
        np_hi = npv.astype(f16)
        np_lo = (npv - np_hi.astype(np.float32)).astype(f16)
        tta = np.empty((K, 2, BPC, N), dtype=f16)
        tta[0:D, 0] = yt.astype(f16).transpose(2, 0, 1)           # [d, b, i]
        tta[0:D, 1] = (-2.0 * yp).astype(f16).transpose(2, 0, 1)  # [d, b, j]
        tta[D, 0] = 1.0
        tta[D, 1] = np_hi
        tta[D + 1, 0] = 1.0
        tta[D + 1, 1] = np_lo
        tta[D + 2, 0] = nt_hi
        tta[D + 2, 1] = 1.0
        tta[D + 3, 0] = nt_lo
        tta[D + 3, 1] = 1.0
        # pair-major: [p, K, 2, 2, N]
        tta_p = np.ascontiguousarray(
            tta.reshape(K, 2, BPC // 2, 2, N).transpose(2, 0, 1, 3, 4)
        )
        maps.append({"tta": tta_p})
    return maps


def kernel(y_true, y_pred):
    y_true = np.asarray(y_true, dtype=np.float32)
    y_pred = np.asarray(y_pred, dtype=np.float32)
    assert y_true.shape == (B, N, D) and y_pred.shape == (B, N, D)

    nc = _get_nc()
    res = run_bass_kernel_spmd(nc, _in_maps(y_true, y_pred), core_ids=list(range(N_CORES)))
    d2 = np.concatenate(
        [res.results[c]["dist2"].astype(np.float32) for c in range(N_CORES)], axis=0
    )
    dist = np.sqrt(np.maximum(d2, 0.0, dtype=np.float32))

    cols = _batched_lsap(dist)  # [B, N]
    matched = np.take_along_axis(dist, cols[:, :, None].astype(np.int64), axis=2)[..., 0]
    loss = np.mean(np.sum(matched.astype(np.float64), axis=1))
    return np.float32(loss)


# revision 25
# speedup vs baseline: 1.1275x; 1.1275x over previous
"""Trainium2 kernel for nn_CustomModel_71227737637112 (Hungarian-matching loss).

reference semantics:
    dist[b,i,j] = || y_true[b,i,:] - y_pred[b,j,:] ||_2          [B=64, N=128, N]
    col = linear_sum_assignment(dist[b])  (host, per batch)
    loss = mean_b sum_i dist[b, i, col[b,i]]

Device part (8 NeuronCores, batch-sharded data parallel, 8 batches/core):
    dist^2 = |yt_i|^2 + |yp_j|^2 - 2 yt_i.yp_j as ONE K=68 bf16 matmul per
    batch.  The host pre-packs the (transposed, d-major) operand image
        lhsT[k,b,i] = [bf16(yt) | 1 | 1 | nt_hi | nt_lo]
        rhs [k,b,j] = [bf16(-2 yp) | np_hi | np_lo | 1 | 1]
    with both norms carried as bf16 value+residual row pairs (~f32
    accuracy).  The device then only does: one contiguous DMA in, 8
    matmuls (PE), 8 PSUM->SBUF copies (DVE/ACT alternating), chunked DMA
    out of dist^2.  Host: sqrt, LSAP (the reference also runs LSAP on host
    via pure_callback), gather + mean.
"""

import numpy as np

import concourse.bacc as bacc
import concourse.bass as bass
import concourse.mybir as mybir
from concourse.bass_utils import run_bass_kernel_spmd
from concourse.tile import TileContext

N_CORES = 8
B, N, D = 64, 128, 64
BPC = B // N_CORES  # batches per core
K = D + 4  # contraction: 64 data rows + np_hi/np_lo + nt_hi/nt_lo vs ones

FP32 = mybir.dt.float32
FP16 = mybir.dt.float16


def _build_nc() -> bass.Bass:
    nc = bacc.Bacc("TRN2", target_bir_lowering=False)
    # Pre-packed transposed operands, half-major for chunked contiguous DMA:
    # [half, k, side (0=lhsT yt, 1=rhs yp), b within half, point]
    tta = nc.dram_tensor("tta", [2, K, 2, BPC // 2, N], FP16, kind="ExternalInput")
    dist2 = nc.dram_tensor("dist2", [BPC, N, N], FP16, kind="ExternalOutput")

    with TileContext(nc) as tc:
        with (
            tc.tile_pool(name="sbuf", bufs=1) as sbuf,
            tc.tile_pool(name="psum_mm", bufs=4, space="PSUM") as psum_mm,
        ):
            tt = sbuf.tile([K, 2, BPC, N], FP16)
            d2 = sbuf.tile([N, BPC, N], FP16)

            # Input split across BOTH physical HWDGE rings (SP + ACT) so the
            # two halves transfer in parallel.
            tt_half = tt[:, :, :, :].rearrange("k s (h c) n -> h k s c n", c=BPC // 2)
            nc.sync.dma_start(out=tt_half[0], in_=tta[0])
            nc.scalar.dma_start(out=tt_half[1], in_=tta[1])

            dist_r = dist2[:, :, :].rearrange("b i j -> i b j")
            for b in range(BPC):
                mm = psum_mm.tile([N, N], FP32, tag="mm")
                nc.tensor.matmul(
                    mm[:, :], tt[:, 0, b, :], tt[:, 1, b, :], start=True, stop=True
                )
                # PSUM -> SBUF f16 copies all on DVE (no ACT compute => no
                # ACT table load; ACT sequencer only issues output DMAs)
                nc.vector.tensor_copy(d2[:, b, :], mm[:, :])
                # output in 2-batch chunks, alternating the two HWDGE rings
                if b % 2 == 1:
                    eng = nc.sync if (b // 2) % 2 == 0 else nc.scalar
                    eng.dma_start(
                        out=dist_r[:, b - 1 : b + 1, :], in_=d2[:, b - 1 : b + 1, :]
                    )

    nc.finalize()
    return nc


_NC_CACHE = None


def _get_nc():
    global _NC_CACHE
    if _NC_CACHE is None:
        _NC_CACHE = _build_nc()
    return _NC_CACHE


# ---------------------------------------------------------------------------
# Host side: batched linear sum assignment (Hungarian).  The reference runs
# this on host through jax.pure_callback; we do the same.  scipy if present,
# else a vectorized Jonker-Volgenant implementation identical to the
# reference algorithm.
# ---------------------------------------------------------------------------


def _lsap_np(cost):
    cost = np.asarray(cost, dtype=np.float64)
    n = cost.shape[0]
    u = np.zeros(n + 1)
    v = np.zeros(n + 1)
    p = np.zeros(n + 1, dtype=np.int64)
    way = np.zeros(n + 1, dtype=np.int64)
    for i in range(1, n + 1):
        p[0] = i
        j0 = 0
        minv = np.full(n + 1, np.inf)
        used = np.zeros(n + 1, dtype=bool)
        while True:
            used[j0] = True
            i0 = p[j0]
            js = np.nonzero(~used[1:])[0] + 1
            cur = cost[i0 - 1, js - 1] - u[i0] - v[js]
            better = cur < minv[js]
            minv[js] = np.where(better, cur, minv[js])
            way[js] = np.where(better, j0, way[js])
            j1 = js[np.argmin(minv[js])]
            delta = minv[j1]
            u[p[used]] += delta
            v[used] -= delta
            minv[~used] -= delta
            j0 = j1
            if p[j0] == 0:
                break
        while j0 != 0:
            j1 = way[j0]
            p[j0] = p[j1]
            j0 = j1
    col_of_row = np.zeros(n, dtype=np.int32)
    for j in range(1, n + 1):
        if p[j] > 0:
            col_of_row[p[j] - 1] = j - 1
    return col_of_row


def _batched_lsap(dists):
    try:
        from scipy.optimize import linear_sum_assignment

        cols = np.empty((dists.shape[0], dists.shape[1]), dtype=np.int32)
        for b in range(dists.shape[0]):
            _, c = linear_sum_assignment(dists[b].astype(np.float64))
            cols[b] = c.astype(np.int32)
        return cols
    except Exception:
        return np.stack([_lsap_np(d) for d in dists]).astype(np.int32)


def _in_maps(y_true, y_pred):
    f16 = np.float16
    maps = []
    for c in range(N_CORES):
        yt = y_true[c * BPC : (c + 1) * BPC]  # [8, 128, 64] f32
        yp = y_pred[c * BPC : (c + 1) * BPC]
        nt = (yt.astype(np.float64) ** 2).sum(-1).astype(np.float32)  # [8, 128]
        npv = (yp.astype(np.float64) ** 2).sum(-1).astype(np.float32)
        nt_hi = nt.astype(f16)
        nt_lo = (nt - nt_hi.astype(np.float32)).astype(f16)
        np_hi = npv.astype(f16)
        np_lo = (npv - np_hi.astype(np.float32)).astype(f16)
        tta = np.empty((K, 2, BPC, N), dtype=f16)
        tta[0:D, 0] = yt.astype(f16).transpose(2, 0, 1)           # [d, b, i]
        tta[0:D, 1] = (-2.0 * yp).astype(f16).transpose(2, 0, 1)  # [d, b, j]
        tta[D, 0] = 1.0
        tta[D, 1] = np_hi
        tta[D + 1, 0] = 1.0
        tta[D + 1, 1] = np_lo
        tta[D + 2, 0] = nt_hi
        tta[D + 2, 1] = 1.0
        tta[D + 3, 0] = nt_lo
        tta[D + 3, 1] = 1.0
        # half-major: [2, K, 2, BPC//2, N]
        tta_p = np.ascontiguousarray(
            tta.reshape(K, 2, 2, BPC // 2, N).transpose(2, 0, 1, 3, 4)
        )
        maps.append({"tta": tta_p})
    return maps


def kernel(y_true, y_pred):
    y_true = np.asarray(y_true, dtype=np.float32)
    y_pred = np.asarray(y_pred, dtype=np.float32)
    assert y_true.shape == (B, N, D) and y_pred.shape == (B, N, D)

    nc = _get_nc()
    res = run_bass_kernel_spmd(nc, _in_maps(y_true, y_pred), core_ids=list(range(N_CORES)))
    d2 = np.concatenate(
        [res.results[c]["dist2"].astype(np.float32) for c in range(N_CORES)], axis=0
    )
    dist = np.sqrt(np.maximum(d2, 0.0, dtype=np.float32))

    cols = _batched_lsap(dist)  # [B, N]
    matched = np.take_along_axis(dist, cols[:, :, None].astype(np.int64), axis=2)[..., 0]
    loss = np.mean(np.sum(matched.astype(np.float64), axis=1))
    return np.float32(loss)


# revision 29
# speedup vs baseline: 1.1936x; 1.0586x over previous
"""Trainium2 kernel for nn_CustomModel_71227737637112 (Hungarian-matching loss).

reference semantics:
    dist[b,i,j] = || y_true[b,i,:] - y_pred[b,j,:] ||_2          [B=64, N=128, N]
    col = linear_sum_assignment(dist[b])  (host, per batch)
    loss = mean_b sum_i dist[b, i, col[b,i]]

Device part (8 NeuronCores, batch-sharded data parallel, 8 batches/core):
    dist^2 = |yt_i|^2 + |yp_j|^2 - 2 yt_i.yp_j as ONE K=68 bf16 matmul per
    batch.  The host pre-packs the (transposed, d-major) operand image
        lhsT[k,b,i] = [bf16(yt) | 1 | 1 | nt_hi | nt_lo]
        rhs [k,b,j] = [bf16(-2 yp) | np_hi | np_lo | 1 | 1]
    with both norms carried as bf16 value+residual row pairs (~f32
    accuracy).  The device then only does: one contiguous DMA in, 8
    matmuls (PE), 8 PSUM->SBUF copies (DVE/ACT alternating), chunked DMA
    out of dist^2.  Host: sqrt, LSAP (the reference also runs LSAP on host
    via pure_callback), gather + mean.
"""

import numpy as np

import concourse.bacc as bacc
import concourse.bass as bass
import concourse.mybir as mybir
from concourse.bass_utils import run_bass_kernel_spmd
from concourse.tile import TileContext

N_CORES = 8
B, N, D = 64, 128, 64
BPC = B // N_CORES  # batches per core
K = D + 4  # contraction: 64 data rows + np_hi/np_lo + nt_hi/nt_lo vs ones

FP32 = mybir.dt.float32
FP16 = mybir.dt.float16


def _build_nc() -> bass.Bass:
    nc = bacc.Bacc("TRN2", target_bir_lowering=False)
    # Full-128-partition operand images (full SBUF-port DMA bandwidth):
    # xa[k, c*N + i], k<64: yt d-row k of batch c; k>=64: batch c+4.
    # cols 512:520 = nt_hi per batch, 520:528 = nt_lo (epilogue scalars).
    # ya: same for bf16(-2*yp), no extra cols.
    # npw[r, bank, c*N + j]: r0 = np_hi, r1 = np_lo for batch bank*4+c.
    HB = BPC // 2  # 4 batches per partition-block
    xa = nc.dram_tensor("xa", [N, HB * N + 2 * BPC], FP16, kind="ExternalInput")
    ya = nc.dram_tensor("ya", [N, HB * N], FP16, kind="ExternalInput")
    npw = nc.dram_tensor("npw", [2, 2, HB * N], FP16, kind="ExternalInput")
    dist2 = nc.dram_tensor("dist2", [BPC, N, N], FP16, kind="ExternalOutput")

    with TileContext(nc) as tc:
        with (
            tc.tile_pool(name="consts", bufs=1) as consts,
            tc.tile_pool(name="sbuf", bufs=1) as sbuf,
            tc.tile_pool(name="psum_mm", bufs=2, space="PSUM") as psum_mm,
        ):
            ones2 = consts.tile([2, N], FP16)
            nc.vector.memset(ones2[:, :], 1.0)

            xs = sbuf.tile([N, HB * N + 2 * BPC], FP16)
            ys = sbuf.tile([N, HB * N], FP16)
            nps = sbuf.tile([2, 2, HB * N], FP16)
            d2 = sbuf.tile([N, BPC, N], FP16)

            # np rows first (gates the bank-level aug matmuls), then the two
            # big operand images split across the two physical HWDGE rings.
            nc.sync.dma_start(out=nps[:, :, :], in_=npw[:, :, :])
            nc.sync.dma_start(out=xs[:, :], in_=xa[:, :])
            nc.scalar.dma_start(out=ys[:, :], in_=ya[:, :])

            dist_r = dist2[:, :, :].rearrange("b i j -> i b j")
            for bank in range(2):
                mm = psum_mm.tile([N, HB * N], FP32, tag="mm")
                # np_hi[j] + np_lo[j] broadcast over i via a K=2 matmul
                nc.tensor.matmul(
                    mm[:, :], ones2[:, :], nps[:, bank, :],
                    start=True, stop=False, skip_group_check=True,
                )
                for c in range(HB):
                    b = bank * HB + c
                    base = 0 if bank == 0 else D
                    nc.tensor.matmul(
                        mm[:, c * N : (c + 1) * N],
                        xs[base : base + D, c * N : (c + 1) * N],
                        ys[base : base + D, c * N : (c + 1) * N],
                        start=False, stop=True, skip_group_check=True,
                        tile_position=(base, 0),
                    )
                    # epilogue: + nt[i] (f32 bit-packed in the f16 image),
                    # f32 psum -> f16 sbuf
                    nt_ap = xs[:, HB * N + 2 * b : HB * N + 2 * b + 2].bitcast(FP32)
                    nc.vector.tensor_scalar(
                        d2[:, b, :],
                        mm[:, c * N : (c + 1) * N],
                        nt_ap,
                        None,
                        op0=mybir.AluOpType.add,
                    )
                    # output in 2-batch chunks, alternating the HWDGE rings
                    if b % 2 == 1:
                        eng = nc.sync if (b // 2) % 2 == 0 else nc.scalar
                        eng.dma_start(
                            out=dist_r[:, b - 1 : b + 1, :],
                            in_=d2[:, b - 1 : b + 1, :],
                        )

    nc.finalize()
    return nc


_NC_CACHE = None


def _get_nc():
    global _NC_CACHE
    if _NC_CACHE is None:
        _NC_CACHE = _build_nc()
    return _NC_CACHE


# ---------------------------------------------------------------------------
# Host side: batched linear sum assignment (Hungarian).  The reference runs
# this on host through jax.pure_callback; we do the same.  scipy if present,
# else a vectorized Jonker-Volgenant implementation identical to the
# reference algorithm.
# ---------------------------------------------------------------------------


def _lsap_np(cost):
    cost = np.asarray(cost, dtype=np.float64)
    n = cost.shape[0]
    u = np.zeros(n + 1)
    v = np.zeros(n + 1)
    p = np.zeros(n + 1, dtype=np.int64)
    way = np.zeros(n + 1, dtype=np.int64)
    for i in range(1, n + 1):
        p[0] = i
        j0 = 0
        minv = np.full(n + 1, np.inf)
        used = np.zeros(n + 1, dtype=bool)
        while True:
            used[j0] = True
            i0 = p[j0]
            js = np.nonzero(~used[1:])[0] + 1
            cur = cost[i0 - 1, js - 1] - u[i0] - v[js]
            better = cur < minv[js]
            minv[js] = np.where(better, cur, minv[js])
            way[js] = np.where(better, j0, way[js])
            j1 = js[np.argmin(minv[js])]
            delta = minv[j1]
            u[p[used]] += delta
            v[used] -= delta
            minv[~used] -= delta
            j0 = j1
            if p[j0] == 0:
                break
        while j0 != 0:
            j1 = way[j0]
            p[j0] = p[j1]
            j0 = j1
    col_of_row = np.zeros(n, dtype=np.int32)
    for j in range(1, n + 1):
        if p[j] > 0:
            col_of_row[p[j] - 1] = j - 1
    return col_of_row


def _batched_lsap(dists):
    try:
        from scipy.optimize import linear_sum_assignment

        cols = np.empty((dists.shape[0], dists.shape[1]), dtype=np.int32)
        for b in range(dists.shape[0]):
            _, c = linear_sum_assignment(dists[b].astype(np.float64))
            cols[b] = c.astype(np.int32)
        return cols
    except Exception:
        return np.stack([_lsap_np(d) for d in dists]).astype(np.int32)


def _in_maps(y_true, y_pred):
    f16 = np.float16
    HB = BPC // 2
    maps = []
    for c in range(N_CORES):
        yt = y_true[c * BPC : (c + 1) * BPC]  # [8, 128, 64] f32
        yp = y_pred[c * BPC : (c + 1) * BPC]
        nt = (yt.astype(np.float64) ** 2).sum(-1).astype(np.float32)  # [8, 128]
        npv = (yp.astype(np.float64) ** 2).sum(-1).astype(np.float32)
        np_hi = npv.astype(f16)
        np_lo = (npv - np_hi.astype(np.float32)).astype(f16)

        ytT = yt.astype(f16).transpose(2, 0, 1)           # [d, b, i]
        ypT = (-2.0 * yp).astype(f16).transpose(2, 0, 1)  # [d, b, j]

        xa = np.empty((N, HB * N + 2 * BPC), dtype=f16)
        ya = np.empty((N, HB * N), dtype=f16)
        # batches 0..3 on partitions 0:64, 4..7 on 64:128
        xa[0:D, 0 : HB * N] = ytT[:, 0:HB].reshape(D, HB * N)
        xa[D:N, 0 : HB * N] = ytT[:, HB:BPC].reshape(D, HB * N)
        # nt as f32, bit-packed into pairs of f16 slots: [128, 8] f32
        xa[:, HB * N :] = nt.T.copy().view(f16)
        ya[0:D] = ypT[:, 0:HB].reshape(D, HB * N)
        ya[D:N] = ypT[:, HB:BPC].reshape(D, HB * N)

        npw = np.empty((2, 2, HB * N), dtype=f16)
        npw[0] = np_hi.reshape(2, HB * N)
        npw[1] = np_lo.reshape(2, HB * N)

        maps.append(
            {
                "xa": np.ascontiguousarray(xa),
                "ya": np.ascontiguousarray(ya),
                "npw": np.ascontiguousarray(npw),
            }
        )
    return maps


def kernel(y_true, y_pred):
    y_true = np.asarray(y_true, dtype=np.float32)
    y_pred = np.asarray(y_pred, dtype=np.float32)
    assert y_true.shape == (B, N, D) and y_pred.shape == (B, N, D)

    nc = _get_nc()
    res = run_bass_kernel_spmd(nc, _in_maps(y_true, y_pred), core_ids=list(range(N_CORES)))
    d2 = np.concatenate(
        [res.results[c]["dist2"].astype(np.float32) for c in range(N_CORES)], axis=0
    )
    dist = np.sqrt(np.maximum(d2, 0.0, dtype=np.float32))

    cols = _batched_lsap(dist)  # [B, N]
    matched = np.take_along_axis(dist, cols[:, :, None].astype(np.int64), axis=2)[..., 0]
    loss = np.mean(np.sum(matched.astype(np.float64), axis=1))
    return np.float32(loss)


# revision 32
# speedup vs baseline: 1.3497x; 1.1308x over previous
"""Trainium2 kernel for nn_CustomModel_71227737637112 (Hungarian-matching loss).

reference semantics:
    dist[b,i,j] = || y_true[b,i,:] - y_pred[b,j,:] ||_2          [B=64, N=128, N]
    col = linear_sum_assignment(dist[b])  (host, per batch)
    loss = mean_b sum_i dist[b, i, col[b,i]]

Device part (8 NeuronCores, batch-sharded data parallel, 8 batches/core):
    dist^2 = |yt_i|^2 + |yp_j|^2 - 2 yt_i.yp_j as ONE K=68 bf16 matmul per
    batch.  The host pre-packs the (transposed, d-major) operand image
        lhsT[k,b,i] = [bf16(yt) | 1 | 1 | nt_hi | nt_lo]
        rhs [k,b,j] = [bf16(-2 yp) | np_hi | np_lo | 1 | 1]
    with both norms carried as bf16 value+residual row pairs (~f32
    accuracy).  The device then only does: one contiguous DMA in, 8
    matmuls (PE), 8 PSUM->SBUF copies (DVE/ACT alternating), chunked DMA
    out of dist^2.  Host: sqrt, LSAP (the reference also runs LSAP on host
    via pure_callback), gather + mean.
"""

import numpy as np

import concourse.bacc as bacc
import concourse.bass as bass
import concourse.mybir as mybir
from concourse.bass_utils import run_bass_kernel_spmd
from concourse.tile import TileContext

N_CORES = 8
B, N, D = 64, 128, 64
BPC = B // N_CORES  # batches per core
K = D + 4  # contraction: 64 data rows + np_hi/np_lo + nt_hi/nt_lo vs ones

FP32 = mybir.dt.float32
FP16 = mybir.dt.float16


def _build_nc() -> bass.Bass:
    nc = bacc.Bacc("TRN2", target_bir_lowering=False)
    # Full-128-partition operand images (full SBUF-port DMA bandwidth):
    # xa[k, c*N + i], k<64: yt d-row k of batch c; k>=64: batch c+4.
    # ya: same layout for f16(-2*yp).
    # The device computes only -2G; the norms nt[i] + np[j] are added on
    # the host (cheap numpy broadcast) after gathering.
    HB = BPC // 2  # 4 batches per partition-block
    xa = nc.dram_tensor("xa", [N, HB * N], FP16, kind="ExternalInput")
    ya = nc.dram_tensor("ya", [N, HB * N], FP16, kind="ExternalInput")
    dist2 = nc.dram_tensor("dist2", [BPC, N, N], FP16, kind="ExternalOutput")

    with TileContext(nc) as tc:
        with (
            tc.tile_pool(name="sbuf", bufs=1) as sbuf,
            tc.tile_pool(name="psum_mm", bufs=4, space="PSUM") as psum_mm,
        ):
            xs = sbuf.tile([N, HB * N], FP16)
            ys = sbuf.tile([N, HB * N], FP16)
            d2 = sbuf.tile([N, BPC, N], FP16)

            # one big operand image per physical HWDGE ring, in parallel
            nc.sync.dma_start(out=xs[:, :], in_=xa[:, :])
            nc.scalar.dma_start(out=ys[:, :], in_=ya[:, :])

            dist_r = dist2[:, :, :].rearrange("b i j -> i b j")
            for b in range(BPC):
                bank, c = divmod(b, HB)
                base = 0 if bank == 0 else D
                mm = psum_mm.tile([N, N], FP32, tag="mm")
                nc.tensor.matmul(
                    mm[:, :],
                    xs[base : base + D, c * N : (c + 1) * N],
                    ys[base : base + D, c * N : (c + 1) * N],
                    start=True, stop=True,
                    tile_position=(base, 0),
                )
                nc.vector.tensor_copy(d2[:, b, :], mm[:, :])
                # output in 2-batch chunks, alternating the HWDGE rings
                if b % 2 == 1:
                    eng = nc.sync if (b // 2) % 2 == 0 else nc.scalar
                    eng.dma_start(
                        out=dist_r[:, b - 1 : b + 1, :],
                        in_=d2[:, b - 1 : b + 1, :],
                    )

    nc.finalize()
    return nc


_NC_CACHE = None


def _get_nc():
    global _NC_CACHE
    if _NC_CACHE is None:
        _NC_CACHE = _build_nc()
    return _NC_CACHE


# ---------------------------------------------------------------------------
# Host side: batched linear sum assignment (Hungarian).  The reference runs
# this on host through jax.pure_callback; we do the same.  scipy if present,
# else a vectorized Jonker-Volgenant implementation identical to the
# reference algorithm.
# ---------------------------------------------------------------------------


def _lsap_np(cost):
    cost = np.asarray(cost, dtype=np.float64)
    n = cost.shape[0]
    u = np.zeros(n + 1)
    v = np.zeros(n + 1)
    p = np.zeros(n + 1, dtype=np.int64)
    way = np.zeros(n + 1, dtype=np.int64)
    for i in range(1, n + 1):
        p[0] = i
        j0 = 0
        minv = np.full(n + 1, np.inf)
        used = np.zeros(n + 1, dtype=bool)
        while True:
            used[j0] = True
            i0 = p[j0]
            js = np.nonzero(~used[1:])[0] + 1
            cur = cost[i0 - 1, js - 1] - u[i0] - v[js]
            better = cur < minv[js]
            minv[js] = np.where(better, cur, minv[js])
            way[js] = np.where(better, j0, way[js])
            j1 = js[np.argmin(minv[js])]
            delta = minv[j1]
            u[p[used]] += delta
            v[used] -= delta
            minv[~used] -= delta
            j0 = j1
            if p[j0] == 0:
                break
        while j0 != 0:
            j1 = way[j0]
            p[j0] = p[j1]
            j0 = j1
    col_of_row = np.zeros(n, dtype=np.int32)
    for j in range(1, n + 1):
        if p[j] > 0:
            col_of_row[p[j] - 1] = j - 1
    return col_of_row


def _batched_lsap(dists):
    try:
        from scipy.optimize import linear_sum_assignment

        cols = np.empty((dists.shape[0], dists.shape[1]), dtype=np.int32)
        for b in range(dists.shape[0]):
            _, c = linear_sum_assignment(dists[b].astype(np.float64))
            cols[b] = c.astype(np.int32)
        return cols
    except Exception:
        return np.stack([_lsap_np(d) for d in dists]).astype(np.int32)


def _in_maps(y_true, y_pred):
    f16 = np.float16
    HB = BPC // 2
    maps = []
    for c in range(N_CORES):
        yt = y_true[c * BPC : (c + 1) * BPC]  # [8, 128, 64] f32
        yp = y_pred[c * BPC : (c + 1) * BPC]

        ytT = yt.astype(f16).transpose(2, 0, 1)           # [d, b, i]
        ypT = (-2.0 * yp).astype(f16).transpose(2, 0, 1)  # [d, b, j]

        xa = np.empty((N, HB * N), dtype=f16)
        ya = np.empty((N, HB * N), dtype=f16)
        # batches 0..3 on partitions 0:64, 4..7 on 64:128
        xa[0:D] = ytT[:, 0:HB].reshape(D, HB * N)
        xa[D:N] = ytT[:, HB:BPC].reshape(D, HB * N)
        ya[0:D] = ypT[:, 0:HB].reshape(D, HB * N)
        ya[D:N] = ypT[:, HB:BPC].reshape(D, HB * N)

        maps.append(
            {"xa": np.ascontiguousarray(xa), "ya": np.ascontiguousarray(ya)}
        )
    return maps


def kernel(y_true, y_pred):
    y_true = np.asarray(y_true, dtype=np.float32)
    y_pred = np.asarray(y_pred, dtype=np.float32)
    assert y_true.shape == (B, N, D) and y_pred.shape == (B, N, D)

    nc = _get_nc()
    res = run_bass_kernel_spmd(nc, _in_maps(y_true, y_pred), core_ids=list(range(N_CORES)))
    # device returned -2G; add the norms on host
    m2g = np.concatenate(
        [res.results[c]["dist2"].astype(np.float32) for c in range(N_CORES)], axis=0
    )
    nt = (y_true.astype(np.float64) ** 2).sum(-1).astype(np.float32)  # [B, N]
    npv = (y_pred.astype(np.float64) ** 2).sum(-1).astype(np.float32)
    d2 = m2g + nt[:, :, None] + npv[:, None, :]
    dist = np.sqrt(np.maximum(d2, 0.0, dtype=np.float32))

    cols = _batched_lsap(dist)  # [B, N]
    matched = np.take_along_axis(dist, cols[:, :, None].astype(np.int64), axis=2)[..., 0]
    loss = np.mean(np.sum(matched.astype(np.float64), axis=1))
    return np.float32(loss)


# revision 35
# speedup vs baseline: 1.3686x; 1.0140x over previous
"""Trainium2 kernel for nn_CustomModel_71227737637112 (Hungarian-matching loss).

reference semantics:
    dist[b,i,j] = || y_true[b,i,:] - y_pred[b,j,:] ||_2          [B=64, N=128, N]
    col = linear_sum_assignment(dist[b])  (host, per batch)
    loss = mean_b sum_i dist[b, i, col[b,i]]

Device part (8 NeuronCores, batch-sharded data parallel, 8 batches/core):
    dist^2 = |yt_i|^2 + |yp_j|^2 - 2 yt_i.yp_j as ONE K=68 bf16 matmul per
    batch.  The host pre-packs the (transposed, d-major) operand image
        lhsT[k,b,i] = [bf16(yt) | 1 | 1 | nt_hi | nt_lo]
        rhs [k,b,j] = [bf16(-2 yp) | np_hi | np_lo | 1 | 1]
    with both norms carried as bf16 value+residual row pairs (~f32
    accuracy).  The device then only does: one contiguous DMA in, 8
    matmuls (PE), 8 PSUM->SBUF copies (DVE/ACT alternating), chunked DMA
    out of dist^2.  Host: sqrt, LSAP (the reference also runs LSAP on host
    via pure_callback), gather + mean.
"""

import numpy as np

import concourse.bacc as bacc
import concourse.bass as bass
import concourse.mybir as mybir
from concourse.bass_utils import run_bass_kernel_spmd
from concourse.tile import TileContext

N_CORES = 8
B, N, D = 64, 128, 64
BPC = B // N_CORES  # batches per core
K = D + 4  # contraction: 64 data rows + np_hi/np_lo + nt_hi/nt_lo vs ones

FP32 = mybir.dt.float32
FP16 = mybir.dt.float16


def _build_nc() -> bass.Bass:
    nc = bacc.Bacc("TRN2", target_bir_lowering=False)
    # Full-128-partition operand images (full SBUF-port DMA bandwidth):
    # xa[k, c*N + i], k<64: yt d-row k of batch c; k>=64: batch c+4.
    # ya: same layout for f16(-2*yp).
    # The device computes only -2G; the norms nt[i] + np[j] are added on
    # the host (cheap numpy broadcast) after gathering.
    HB = BPC // 2  # 4 batches per partition-block
    xa = nc.dram_tensor("xa", [N, HB * N], FP16, kind="ExternalInput")
    ya = nc.dram_tensor("ya", [N, HB * N], FP16, kind="ExternalInput")
    # i-major output (matches the SBUF image -> big contiguous descriptors);
    # the host transposes back to [b, i, j]
    dist2 = nc.dram_tensor("dist2", [N, BPC, N], FP16, kind="ExternalOutput")

    with TileContext(nc) as tc:
        with (
            tc.tile_pool(name="sbuf", bufs=1) as sbuf,
            tc.tile_pool(name="psum_mm", bufs=4, space="PSUM") as psum_mm,
        ):
            xs = sbuf.tile([N, HB * N], FP16)
            ys = sbuf.tile([N, HB * N], FP16)
            d2 = sbuf.tile([N, BPC, N], FP16)

            # column-split input chunks, crossed over the two HWDGE rings:
            # after the first chunk of each ring, batches {0,1,4,5} can run.
            HC = HB * N // 2
            nc.sync.dma_start(out=xs[:, 0:HC], in_=xa[:, 0:HC])
            nc.scalar.dma_start(out=ys[:, 0:HC], in_=ya[:, 0:HC])
            nc.sync.dma_start(out=ys[:, HC:], in_=ya[:, HC:])
            nc.scalar.dma_start(out=xs[:, HC:], in_=xa[:, HC:])

            # batch pairs ordered by chunk arrival
            for pi, (b0, b1) in enumerate([(0, 1), (4, 5), (2, 3), (6, 7)]):
                bank, c0 = divmod(b0, HB)
                base = 0 if bank == 0 else D
                mm = psum_mm.tile([N, 2, N], FP32, tag="mm")
                for k, b in enumerate((b0, b1)):
                    c = c0 + k
                    nc.tensor.matmul(
                        mm[:, k, :],
                        xs[base : base + D, c * N : (c + 1) * N],
                        ys[base : base + D, c * N : (c + 1) * N],
                        start=True, stop=True, skip_group_check=True,
                        tile_position=(base, 0),
                    )
                # one fused 2-batch PSUM -> SBUF f16 cast
                nc.vector.tensor_copy(d2[:, b0 : b0 + 2, :], mm[:, :, :])
                # 2-batch output chunk, alternating the HWDGE rings
                eng = nc.sync if pi % 2 == 0 else nc.scalar
                eng.dma_start(
                    out=dist2[:, b0 : b0 + 2, :], in_=d2[:, b0 : b0 + 2, :]
                )

    nc.finalize()
    return nc


_NC_CACHE = None


def _get_nc():
    global _NC_CACHE
    if _NC_CACHE is None:
        _NC_CACHE = _build_nc()
    return _NC_CACHE


# ---------------------------------------------------------------------------
# Host side: batched linear sum assignment (Hungarian).  The reference runs
# this on host through jax.pure_callback; we do the same.  scipy if present,
# else a vectorized Jonker-Volgenant implementation identical to the
# reference algorithm.
# ---------------------------------------------------------------------------


def _lsap_np(cost):
    cost = np.asarray(cost, dtype=np.float64)
    n = cost.shape[0]
    u = np.zeros(n + 1)
    v = np.zeros(n + 1)
    p = np.zeros(n + 1, dtype=np.int64)
    way = np.zeros(n + 1, dtype=np.int64)
    for i in range(1, n + 1):
        p[0] = i
        j0 = 0
        minv = np.full(n + 1, np.inf)
        used = np.zeros(n + 1, dtype=bool)
        while True:
            used[j0] = True
            i0 = p[j0]
            js = np.nonzero(~used[1:])[0] + 1
            cur = cost[i0 - 1, js - 1] - u[i0] - v[js]
            better = cur < minv[js]
            minv[js] = np.where(better, cur, minv[js])
            way[js] = np.where(better, j0, way[js])
            j1 = js[np.argmin(minv[js])]
            delta = minv[j1]
            u[p[used]] += delta
            v[used] -= delta
            minv[~used] -= delta
            j0 = j1
            if p[j0] == 0:
                break
        while j0 != 0:
            j1 = way[j0]
            p[j0] = p[j1]
            j0 = j1
    col_of_row = np.zeros(n, dtype=np.int32)
    for j in range(1, n + 1):
        if p[j] > 0:
            col_of_row[p[j] - 1] = j - 1
    return col_of_row


def _batched_lsap(dists):
    try:
        from scipy.optimize import linear_sum_assignment

        cols = np.empty((dists.shape[0], dists.shape[1]), dtype=np.int32)
        for b in range(dists.shape[0]):
            _, c = linear_sum_assignment(dists[b].astype(np.float64))
            cols[b] = c.astype(np.int32)
        return cols
    except Exception:
        return np.stack([_lsap_np(d) for d in dists]).astype(np.int32)


def _in_maps(y_true, y_pred):
    f16 = np.float16
    HB = BPC // 2
    maps = []
    for c in range(N_CORES):
        yt = y_true[c * BPC : (c + 1) * BPC]  # [8, 128, 64] f32
        yp = y_pred[c * BPC : (c + 1) * BPC]

        ytT = yt.astype(f16).transpose(2, 0, 1)           # [d, b, i]
        ypT = (-2.0 * yp).astype(f16).transpose(2, 0, 1)  # [d, b, j]

        xa = np.empty((N, HB * N), dtype=f16)
        ya = np.empty((N, HB * N), dtype=f16)
        # batches 0..3 on partitions 0:64, 4..7 on 64:128
        xa[0:D] = ytT[:, 0:HB].reshape(D, HB * N)
        xa[D:N] = ytT[:, HB:BPC].reshape(D, HB * N)
        ya[0:D] = ypT[:, 0:HB].reshape(D, HB * N)
        ya[D:N] = ypT[:, HB:BPC].reshape(D, HB * N)

        maps.append(
            {"xa": np.ascontiguousarray(xa), "ya": np.ascontiguousarray(ya)}
        )
    return maps


def kernel(y_true, y_pred):
    y_true = np.asarray(y_true, dtype=np.float32)
    y_pred = np.asarray(y_pred, dtype=np.float32)
    assert y_true.shape == (B, N, D) and y_pred.shape == (B, N, D)

    nc = _get_nc()
    res = run_bass_kernel_spmd(nc, _in_maps(y_true, y_pred), core_ids=list(range(N_CORES)))
    # device returned -2G in [i, b, j] layout; transpose + add norms on host
    m2g = np.concatenate(
        [
            res.results[c]["dist2"].transpose(1, 0, 2).astype(np.float32)
            for c in range(N_CORES)
        ],
        axis=0,
    )
    nt = (y_true.astype(np.float64) ** 2).sum(-1).astype(np.float32)  # [B, N]
    npv = (y_pred.astype(np.float64) ** 2).sum(-1).astype(np.float32)
    d2 = m2g + nt[:, :, None] + npv[:, None, :]
    dist = np.sqrt(np.maximum(d2, 0.0, dtype=np.float32))

    cols = _batched_lsap(dist)  # [B, N]
    matched = np.take_along_axis(dist, cols[:, :, None].astype(np.int64), axis=2)[..., 0]
    loss = np.mean(np.sum(matched.astype(np.float64), axis=1))
    return np.float32(loss)
